# revision 32
# baseline (speedup 1.0000x reference)
"""Trainium2 Bass kernel for PixelSNAIL-style strict-causal attention.

Problem: query/key/value [B=4, H=64, W=64, C=256] fp32.
  S = 4096 tokens per batch; scores = (Q K^T)/16 with strict causal mask
  (position i attends to j < i); out = softmax(scores) @ V (row 0 -> 0).

Strategy (8 NeuronCores):
  - 2 cores per batch: context-parallel split of the key/value blocks by
    parity (core h owns k-blocks h, h+2, ..., h+30). Every core runs the
    IDENTICAL program (SPMD) over all 32 query blocks of its batch.
  - No max-subtraction in softmax (scores ~ N(0,1), exp is safe), so
    per-core partial numerators/denominators combine exactly on host.
  - Q/K/V are pre-converted to bf16 on host (halves DMA, full-rate PE,
    hidden weight loads); exp output (probs) is bf16 too. PSUM stays f32.
  - Host pre-transposes Q and K (c-major) so no on-chip transposes are
    needed; V gets a ones-column appended so the softmax denominator
    accumulates in PSUM alongside the numerator.
  - Software pipelining: the PV matmuls of pair k are issued AFTER the
    scores matmuls of pair k+1, so the exp (ACT) of pair k overlaps PE
    work and the PE never stalls waiting on the activation.
  - The strict-causal/diagonal masking is data-driven: an additive mask
    applied on each q-slot's last position-pair, with the mask+exp chain
    split in halves so it fits inside the pipeline window.

Layout per core (b = core//2, h = core%2):
  qt_in [256, 4096] bf16 = Q[b]^T
  kt_in [256, 2048] bf16 = K[b][blocks h::2]^T
  v_in  [2048, 258] bf16 = V[b][blocks h::2] ++ ones column
  m_in  [128, 768]  f32  = additive mask for the last position-pair
  o_out [4096, 258] f32  = partial (numerator ++ denominator)
"""

import numpy as np
import ml_dtypes

BF16 = np.dtype(ml_dtypes.bfloat16)

B = 4
S = 4096          # 64*64 tokens per batch
C = 256
NBLK = 32         # 128-row k blocks per batch
NPOS = 16         # k blocks per core (parity split)
NSLOT = 8         # q slots of 512 rows
SCALE = 1.0 / 16.0
NEG = -1.0e30

_CACHE = {}


def _build_nc():
    import concourse.bacc as bacc
    import concourse.mybir as mybir
    import concourse.tile as tile

    F32 = mybir.dt.float32
    BF = mybir.dt.bfloat16

    nc = bacc.Bacc("TRN2", target_bir_lowering=False, debug=False, num_devices=8)
    qt_in = nc.dram_tensor("qt_in", [C, S], BF, kind="ExternalInput").ap()
    kt_in = nc.dram_tensor("kt_in", [C, NPOS * 128], BF, kind="ExternalInput").ap()
    v_in = nc.dram_tensor("v_in", [NPOS * 128, 258], BF, kind="ExternalInput").ap()
    m_in = nc.dram_tensor("m_in", [128, 512], F32, kind="ExternalInput").ap()
    o_out = nc.dram_tensor("o_out", [S, 258], BF, kind="ExternalOutput").ap()

    with tile.TileContext(nc) as tc:
        with (
            tc.tile_pool(name="const", bufs=1) as const,
            tc.tile_pool(name="pt", bufs=3) as ptp,
            tc.tile_pool(name="osb", bufs=2) as osbp,
            tc.tile_pool(name="st", bufs=2, space="PSUM") as stp,
            tc.tile_pool(name="op", bufs=4, space="PSUM") as opp,
        ):
            # PE warmup: tiny matmuls on memset data during the DMA
            # preamble so the HAM clock gate / p-state ramps before work.
            wu = const.tile([128, 64], BF, tag="wu")
            nc.gpsimd.memset(wu[:], 0.0)
            wu_ps = stp.tile([128, 512], F32, tag="st", name="wu_ps")
            for _ in range(60):
                nc.tensor.matmul(wu_ps[0:64, 0:64], lhsT=wu[:], rhs=wu[:],
                                 start=True, stop=True)

            qt = [const.tile([128, S], BF, tag=f"qt{c}", name=f"qt{c}") for c in range(2)]
            kt = [
                const.tile([128, NPOS * 128], BF, tag=f"kt{c}", name=f"kt{c}")
                for c in range(2)
            ]
            vsb = const.tile([128, NPOS * 258], BF, tag="v")
            mask = const.tile([128, 512], F32, tag="m")

            # DMA placement: q/k/mask inputs ride sync in consumption order;
            # k pair 0 + v ride gpsimd; the scalar(ACT) queue stays exp-only.
            def qt_dma(c, c0, c1):
                nc.sync.dma_start(qt[c][:, c0:c1], qt_in[c * 128:(c + 1) * 128, c0:c1])

            def kt_dma(c, c0, c1):
                nc.sync.dma_start(kt[c][:, c0:c1], kt_in[c * 128:(c + 1) * 128, c0:c1])

            def v_dma(pos, npos):
                nc.gpsimd.dma_start(
                    vsb[:, pos * 258:(pos + npos) * 258].rearrange(
                        "p (t v) -> p t v", t=npos
                    ),
                    v_in[pos * 128:(pos + npos) * 128, :].rearrange(
                        "(t p) v -> p t v", p=128
                    ),
                )

            # gpsimd queue: k pair 0 first (gates the first real matmul),
            # then v in consumption order.
            for c in range(2):
                nc.gpsimd.dma_start(kt[c][:, 0:256], kt_in[c * 128:(c + 1) * 128, 0:256])
            v_dma(0, 2)
            v_dma(2, 2)
            v_dma(4, 4)
            v_dma(8, 8)
            # sync queue: slot-7 queries first, then k pairs, mask, rest of q.
            for c in range(2):
                qt_dma(c, 7 * 512, 8 * 512)
            for c in range(2):
                kt_dma(c, 256, 1024)                  # k pairs 1-3
            for c in range(2):
                kt_dma(c, 1024, 2048)                 # k pairs 4-7
            nc.sync.dma_start(mask[:], m_in[:])
            for c in range(2):
                qt_dma(c, 6 * 512, 7 * 512)
            for c in range(2):
                qt_dma(c, 4 * 512, 6 * 512)
            for c in range(2):
                qt_dma(c, 0, 4 * 512)

            o_ps = {}
            # Pairs awaiting PV issue: [p, t, pt tile, age]. Non-diag pairs
            # flush one pair after their scores (software pipelining); diag
            # pairs flush after two, giving the mask+exp chain extra slack.
            pending = []

            def issue_pv(p, t, pt):
                last = t == p
                first_it = t == 0
                for jp in range(2):
                    for qs in range(4):
                        if last and jp == 1 and qs < 2:
                            continue
                        loff = jp * 512 + qs * 128
                        if last and jp == 1:
                            loff = 512 + (qs - 2) * 128
                        nc.tensor.matmul(
                            o_ps[p][qs][:],
                            lhsT=pt[:, loff:loff + 128],
                            rhs=vsb[:, (2 * t + jp) * 258:(2 * t + jp + 1) * 258],
                            start=(first_it and jp == 0),
                            stop=(last and jp == (0 if qs < 2 else 1)),
                        )
                if last:
                    # Slot p complete: drain its PSUM O tiles into one bf16
                    # staging tile (GPSIMD cannot read PSUM on TRN2), then a
                    # single DMA. On the final slots the scalar engine (idle
                    # by then) takes half the copies to shorten the tail.
                    ob = osbp.tile([128, 4 * 258], BF, tag="ob", name=f"ob{p}")
                    for qs in range(4):
                        dst = ob[:, qs * 258:(qs + 1) * 258]
                        if p <= 1 and qs >= 2:
                            nc.scalar.copy(dst, o_ps[p][qs][:])
                        else:
                            nc.vector.tensor_copy(dst, o_ps[p][qs][:])
                    nc.sync.dma_start(
                        o_out[p * 512:(p + 1) * 512, :].rearrange(
                            "(q part) v -> part q v", part=128
                        ),
                        ob[:].rearrange("part (q v) -> part q v", q=4),
                    )

            for p in range(NSLOT - 1, -1, -1):
                o_ps[p] = [
                    opp.tile([128, 258], F32, tag="o", name=f"o_ps{p}_{qs}")
                    for qs in range(4)
                ]
                for t in range(p + 1):
                    last = t == p
                    width = 768 if last else 1024
                    st = stp.tile([128, 1024], F32, tag="st", name=f"st{p}_{t}")
                    if p == NSLOT - 1 and t < 3:
                        # keep the PE clock gate open while DMA-paced
                        for _ in range(12 if t == 0 else 6):
                            nc.tensor.matmul(
                                st[0:64, 0:64], lhsT=wu[:], rhs=wu[:],
                                start=True, stop=True,
                            )
                    for jp in range(2):
                        pos = 2 * t + jp
                        qoff = p * 512 + (256 if (last and jp == 1) else 0)
                        n = 256 if (last and jp == 1) else 512
                        for c in range(2):
                            nc.tensor.matmul(
                                st[:, jp * 512:jp * 512 + n],
                                lhsT=kt[c][:, pos * 128:(pos + 1) * 128],
                                rhs=qt[c][:, qoff:qoff + n],
                                start=(c == 0),
                                stop=(c == 1),
                            )
                    pt = ptp.tile([128, 1024], BF, tag="pt", name=f"pt{p}_{t}")
                    if last:
                        # Only 256 columns per jp-half can actually be masked
                        # (qs-blocks at/past the diagonal); mask just those so
                        # the mask+exp chain stays inside the pipeline window.
                        nc.vector.tensor_tensor(
                            st[:, 0:256], st[:, 0:256], mask[:, 0:256],
                            mybir.AluOpType.add,
                        )
                        nc.scalar.activation(
                            pt[:, 0:512], st[:, 0:512],
                            mybir.ActivationFunctionType.Exp, scale=SCALE,
                        )
                        nc.vector.tensor_tensor(
                            st[:, 512:768], st[:, 512:768], mask[:, 256:512],
                            mybir.AluOpType.add,
                        )
                        nc.scalar.activation(
                            pt[:, 512:768], st[:, 512:768],
                            mybir.ActivationFunctionType.Exp, scale=SCALE,
                        )
                    else:
                        nc.scalar.activation(
                            pt[:, :width], st[:, :width],
                            mybir.ActivationFunctionType.Exp, scale=SCALE,
                        )
                    for q in pending:
                        q[3] += 1
                    if pending and pending[0][3] >= (2 if pending[0][1] == pending[0][0] else 1):
                        issue_pv(*pending.pop(0)[:3])
                    pending.append([p, t, pt, 0])
            for q in pending:
                issue_pv(*q[:3])
    nc.compile()
    return nc


def _get_nc():
    if "nc" not in _CACHE:
        _CACHE["nc"] = _build_nc()
    return _CACHE["nc"]


def _make_masks():
    """Additive masks [128, 512] for the last position-pair of each slot.

    Only the q-sub-blocks at/past the diagonal can be masked: st columns
    0:256 (jp=0, qs 0-1) and 512:768 (jp=1, qs 2-3). On the last pair t=p,
    position jp holds k-block 4p + 2*jp + h vs q-sub-block 4p + qs:
      block <  qblock -> fully allowed (0)
      block == qblock -> strict lower-triangular (allowed iff q_local > k_local)
      block >  qblock -> fully blocked (NEG)
    """
    k_loc = np.arange(128)[:, None]
    q_loc = np.arange(128)[None, :]
    strict = np.where(q_loc > k_loc, 0.0, NEG).astype(np.float32)
    zeros = np.zeros((128, 128), np.float32)
    blocked = np.full((128, 128), NEG, np.float32)
    masks = []
    for h in range(2):
        chunks = []
        for jp, qs_list in ((0, (0, 1)), (1, (2, 3))):
            rel = 2 * jp + h  # k-block offset relative to 4p
            for qs in qs_list:
                if rel < qs:
                    chunks.append(zeros)
                elif rel == qs:
                    chunks.append(strict)
                else:
                    chunks.append(blocked)
        masks.append(np.concatenate(chunks, axis=1))
    return masks


def _run(query, key, value, trace=False, trace_cores=None):
    from concourse.bass_utils import run_bass_kernel_spmd

    query = np.ascontiguousarray(np.asarray(query, dtype=np.float32)).reshape(B, S, C)
    key = np.ascontiguousarray(np.asarray(key, dtype=np.float32)).reshape(B, S, C)
    value = np.ascontiguousarray(np.asarray(value, dtype=np.float32)).reshape(B, S, C)

    masks = _make_masks()
    pad = np.zeros((NPOS * 128, 2), np.float32)
    pad[:, 0] = 1.0
    in_maps = []
    for core in range(8):
        b, h = core // 2, core % 2
        k_sel = key[b].reshape(NBLK, 128, C)[h::2].reshape(NPOS * 128, C)
        v_sel = value[b].reshape(NBLK, 128, C)[h::2].reshape(NPOS * 128, C)
        in_maps.append(
            {
                "qt_in": np.ascontiguousarray(query[b].T.astype(BF16)),
                "kt_in": np.ascontiguousarray(k_sel.T.astype(BF16)),
                "v_in": np.ascontiguousarray(
                    np.concatenate([v_sel, pad], axis=1).astype(BF16)
                ),
                "m_in": masks[h],
            }
        )

    nc = _get_nc()
    res = run_bass_kernel_spmd(
        nc,
        in_maps,
        list(range(8)),
        trace=trace,
        trace_cores=trace_cores,
    )

    out = np.empty((B, S, C), np.float32)
    for b in range(B):
        o0 = np.asarray(res.results[2 * b]["o_out"]).astype(np.float64)
        o1 = np.asarray(res.results[2 * b + 1]["o_out"]).astype(np.float64)
        num = o0[:, :C] + o1[:, :C]
        den = o0[:, C] + o1[:, C]
        den = np.where(den == 0.0, 1.0, den)
        out[b] = (num / den[:, None]).astype(np.float32)
    return out.reshape(B, 64, 64, C), res


def kernel(query, key, value):
    out, _ = _run(query, key, value, trace=False)
    return out


# revision 35
# speedup vs baseline: 1.1388x; 1.1388x over previous
"""Trainium2 Bass kernel for PixelSNAIL-style strict-causal attention.

Problem: query/key/value [B=4, H=64, W=64, C=256] fp32.
  S = 4096 tokens per batch; scores = (Q K^T)/16 with strict causal mask
  (position i attends to j < i); out = softmax(scores) @ V (row 0 -> 0).

Strategy (8 NeuronCores):
  - 2 cores per batch: context-parallel split of the key/value blocks by
    parity (core h owns k-blocks h, h+2, ..., h+30). Every core runs the
    IDENTICAL program (SPMD) over all 32 query blocks of its batch.
  - No max-subtraction in softmax (scores ~ N(0,1), exp is safe), so
    per-core partial numerators/denominators combine exactly on host.
  - Q/K/V are pre-converted to bf16 on host (halves DMA, full-rate PE,
    hidden weight loads); exp output (probs) is bf16 too. PSUM stays f32.
  - Host pre-transposes Q and K (c-major) so no on-chip transposes are
    needed; V gets a ones-column appended so the softmax denominator
    accumulates in PSUM alongside the numerator.
  - Software pipelining: the PV matmuls of pair k are issued AFTER the
    scores matmuls of pair k+1, so the exp (ACT) of pair k overlaps PE
    work and the PE never stalls waiting on the activation.
  - The strict-causal/diagonal masking is data-driven: an additive mask
    applied on each q-slot's last position-pair, with the mask+exp chain
    split in halves so it fits inside the pipeline window.

Layout per core (b = core//2, h = core%2):
  qt_in [256, 4096] bf16 = Q[b]^T
  kt_in [256, 2048] bf16 = K[b][blocks h::2]^T
  v_in  [2048, 258] bf16 = V[b][blocks h::2] ++ ones column
  m_in  [128, 768]  f32  = additive mask for the last position-pair
  o_out [4096, 258] f32  = partial (numerator ++ denominator)
"""

import numpy as np
import ml_dtypes

BF16 = np.dtype(ml_dtypes.bfloat16)

B = 4
S = 4096          # 64*64 tokens per batch
C = 256
NBLK = 32         # 128-row k blocks per batch
NPOS = 16         # k blocks per core (parity split)
NSLOT = 8         # q slots of 512 rows
SCALE = 1.0 / 16.0
NEG = -1.0e30

_CACHE = {}


def _build_nc():
    import concourse.bacc as bacc
    import concourse.mybir as mybir
    import concourse.tile as tile

    F32 = mybir.dt.float32
    BF = mybir.dt.bfloat16

    nc = bacc.Bacc("TRN2", target_bir_lowering=False, debug=False, num_devices=8)
    qt_in = nc.dram_tensor("qt_in", [C, S], BF, kind="ExternalInput").ap()
    kt_in = nc.dram_tensor("kt_in", [C, NPOS * 128], BF, kind="ExternalInput").ap()
    v_in = nc.dram_tensor("v_in", [NPOS * 128, 258], BF, kind="ExternalInput").ap()
    m_in = nc.dram_tensor("m_in", [128, 512], F32, kind="ExternalInput").ap()
    o_out = nc.dram_tensor("o_out", [S, 258], BF, kind="ExternalOutput").ap()

    with tile.TileContext(nc) as tc:
        with (
            tc.tile_pool(name="const", bufs=1) as const,
            tc.tile_pool(name="pt", bufs=3) as ptp,
            tc.tile_pool(name="osb", bufs=2) as osbp,
            tc.tile_pool(name="st", bufs=2, space="PSUM") as stp,
            tc.tile_pool(name="op", bufs=4, space="PSUM") as opp,
        ):
            # PE warmup: tiny matmuls on memset data during the DMA
            # preamble so the HAM clock gate / p-state ramps before work.
            wu = const.tile([128, 64], BF, tag="wu")
            nc.gpsimd.memset(wu[:], 0.0)
            wu_ps = stp.tile([128, 512], F32, tag="st", name="wu_ps")
            for _ in range(60):
                nc.tensor.matmul(wu_ps[0:64, 0:64], lhsT=wu[:], rhs=wu[:],
                                 start=True, stop=True)

            qt = [const.tile([128, S], BF, tag=f"qt{c}", name=f"qt{c}") for c in range(2)]
            kt = [
                const.tile([128, NPOS * 128], BF, tag=f"kt{c}", name=f"kt{c}")
                for c in range(2)
            ]
            vsb = const.tile([128, NPOS * 258], BF, tag="v")
            mask = const.tile([128, 512], F32, tag="m")

            # DMA placement: q/k/mask inputs ride sync in consumption order;
            # k pair 0 + v ride gpsimd; the scalar(ACT) queue stays exp-only.
            def qt_dma(c, c0, c1):
                nc.sync.dma_start(qt[c][:, c0:c1], qt_in[c * 128:(c + 1) * 128, c0:c1])

            def kt_dma(c, c0, c1):
                nc.sync.dma_start(kt[c][:, c0:c1], kt_in[c * 128:(c + 1) * 128, c0:c1])

            def v_dma(pos, npos):
                nc.gpsimd.dma_start(
                    vsb[:, pos * 258:(pos + npos) * 258].rearrange(
                        "p (t v) -> p t v", t=npos
                    ),
                    v_in[pos * 128:(pos + npos) * 128, :].rearrange(
                        "(t p) v -> p t v", p=128
                    ),
                )

            # Each DGE ring delivers ~1 transfer/1.2us; sync starts fastest
            # (~9us), gpsimd/scalar ~11.5-12us. Spread the six start-gating
            # transfers across rings by consumption deadline.
            nc.sync.dma_start(kt[0][:, 0:256], kt_in[0:128, 0:256])      # kt0 c0
            qt_dma(0, 7 * 512, 8 * 512)                                  # qt7 c0
            qt_dma(1, 7 * 512, 8 * 512)                                  # qt7 c1
            kt_dma(0, 256, 1024)                                         # kt1-3 c0
            nc.gpsimd.dma_start(kt[1][:, 0:256], kt_in[128:256, 0:256])  # kt0 c1
            nc.scalar.dma_start(kt[1][:, 256:1024],
                                kt_in[128:256, 256:1024])                # kt1-3 c1
            v_dma(0, 2)
            v_dma(2, 2)
            v_dma(4, 4)
            v_dma(8, 8)
            for c in range(2):
                kt_dma(c, 1024, 2048)                 # k pairs 4-7
            nc.sync.dma_start(mask[:], m_in[:])
            for c in range(2):
                qt_dma(c, 6 * 512, 7 * 512)
            for c in range(2):
                qt_dma(c, 4 * 512, 6 * 512)
            for c in range(2):
                qt_dma(c, 0, 4 * 512)

            o_ps = {}
            # Pairs awaiting PV issue: [p, t, pt tile, age]. Non-diag pairs
            # flush one pair after their scores (software pipelining); diag
            # pairs flush after two, giving the mask+exp chain extra slack.
            pending = []

            def issue_pv(p, t, pt):
                last = t == p
                first_it = t == 0
                for jp in range(2):
                    for qs in range(4):
                        if last and jp == 1 and qs < 2:
                            continue
                        loff = jp * 512 + qs * 128
                        if last and jp == 1:
                            loff = 512 + (qs - 2) * 128
                        nc.tensor.matmul(
                            o_ps[p][qs][:],
                            lhsT=pt[:, loff:loff + 128],
                            rhs=vsb[:, (2 * t + jp) * 258:(2 * t + jp + 1) * 258],
                            start=(first_it and jp == 0),
                            stop=(last and jp == (0 if qs < 2 else 1)),
                        )
                if last:
                    # Slot p complete: drain its PSUM O tiles into one bf16
                    # staging tile (GPSIMD cannot read PSUM on TRN2), then a
                    # single DMA. On the final slots the scalar engine (idle
                    # by then) takes half the copies to shorten the tail.
                    ob = osbp.tile([128, 4 * 258], BF, tag="ob", name=f"ob{p}")
                    for qs in range(4):
                        dst = ob[:, qs * 258:(qs + 1) * 258]
                        if p <= 1 and qs >= 2:
                            nc.scalar.copy(dst, o_ps[p][qs][:])
                        else:
                            nc.vector.tensor_copy(dst, o_ps[p][qs][:])
                    nc.sync.dma_start(
                        o_out[p * 512:(p + 1) * 512, :].rearrange(
                            "(q part) v -> part q v", part=128
                        ),
                        ob[:].rearrange("part (q v) -> part q v", q=4),
                    )

            for p in range(NSLOT - 1, -1, -1):
                o_ps[p] = [
                    opp.tile([128, 258], F32, tag="o", name=f"o_ps{p}_{qs}")
                    for qs in range(4)
                ]
                for t in range(p + 1):
                    last = t == p
                    width = 768 if last else 1024
                    st = stp.tile([128, 1024], F32, tag="st", name=f"st{p}_{t}")
                    if p == NSLOT - 1 and t < 3:
                        # keep the PE clock gate open while DMA-paced
                        for _ in range(12 if t == 0 else 6):
                            nc.tensor.matmul(
                                st[0:64, 0:64], lhsT=wu[:], rhs=wu[:],
                                start=True, stop=True,
                            )
                    # On the diag pair, compute the masked jp1 half first so
                    # its mask+exp chain starts while jp0's scores still run.
                    for jp in ((1, 0) if last else (0, 1)):
                        pos = 2 * t + jp
                        qoff = p * 512 + (256 if (last and jp == 1) else 0)
                        n = 256 if (last and jp == 1) else 512
                        for c in range(2):
                            nc.tensor.matmul(
                                st[:, jp * 512:jp * 512 + n],
                                lhsT=kt[c][:, pos * 128:(pos + 1) * 128],
                                rhs=qt[c][:, qoff:qoff + n],
                                start=(c == 0),
                                stop=(c == 1),
                            )
                    pt = ptp.tile([128, 1024], BF, tag="pt", name=f"pt{p}_{t}")
                    if last:
                        # Only 256 columns per jp-half can actually be masked
                        # (qs-blocks at/past the diagonal); mask just those so
                        # the mask+exp chain stays inside the pipeline window.
                        nc.vector.tensor_tensor(
                            st[:, 512:768], st[:, 512:768], mask[:, 256:512],
                            mybir.AluOpType.add,
                        )
                        nc.scalar.activation(
                            pt[:, 512:768], st[:, 512:768],
                            mybir.ActivationFunctionType.Exp, scale=SCALE,
                        )
                        nc.vector.tensor_tensor(
                            st[:, 0:256], st[:, 0:256], mask[:, 0:256],
                            mybir.AluOpType.add,
                        )
                        nc.scalar.activation(
                            pt[:, 0:512], st[:, 0:512],
                            mybir.ActivationFunctionType.Exp, scale=SCALE,
                        )
                    else:
                        nc.scalar.activation(
                            pt[:, :width], st[:, :width],
                            mybir.ActivationFunctionType.Exp, scale=SCALE,
                        )
                    for q in pending:
                        q[3] += 1
                    if pending and pending[0][3] >= (2 if pending[0][1] == pending[0][0] else 1):
                        issue_pv(*pending.pop(0)[:3])
                    pending.append([p, t, pt, 0])
            for q in pending:
                issue_pv(*q[:3])
    nc.compile()
    return nc


def _get_nc():
    if "nc" not in _CACHE:
        _CACHE["nc"] = _build_nc()
    return _CACHE["nc"]


def _make_masks():
    """Additive masks [128, 512] for the last position-pair of each slot.

    Only the q-sub-blocks at/past the diagonal can be masked: st columns
    0:256 (jp=0, qs 0-1) and 512:768 (jp=1, qs 2-3). On the last pair t=p,
    position jp holds k-block 4p + 2*jp + h vs q-sub-block 4p + qs:
      block <  qblock -> fully allowed (0)
      block == qblock -> strict lower-triangular (allowed iff q_local > k_local)
      block >  qblock -> fully blocked (NEG)
    """
    k_loc = np.arange(128)[:, None]
    q_loc = np.arange(128)[None, :]
    strict = np.where(q_loc > k_loc, 0.0, NEG).astype(np.float32)
    zeros = np.zeros((128, 128), np.float32)
    blocked = np.full((128, 128), NEG, np.float32)
    masks = []
    for h in range(2):
        chunks = []
        for jp, qs_list in ((0, (0, 1)), (1, (2, 3))):
            rel = 2 * jp + h  # k-block offset relative to 4p
            for qs in qs_list:
                if rel < qs:
                    chunks.append(zeros)
                elif rel == qs:
                    chunks.append(strict)
                else:
                    chunks.append(blocked)
        masks.append(np.concatenate(chunks, axis=1))
    return masks


def _run(query, key, value, trace=False, trace_cores=None):
    from concourse.bass_utils import run_bass_kernel_spmd

    query = np.ascontiguousarray(np.asarray(query, dtype=np.float32)).reshape(B, S, C)
    key = np.ascontiguousarray(np.asarray(key, dtype=np.float32)).reshape(B, S, C)
    value = np.ascontiguousarray(np.asarray(value, dtype=np.float32)).reshape(B, S, C)

    masks = _make_masks()
    pad = np.zeros((NPOS * 128, 2), np.float32)
    pad[:, 0] = 1.0
    in_maps = []
    for core in range(8):
        b, h = core // 2, core % 2
        k_sel = key[b].reshape(NBLK, 128, C)[h::2].reshape(NPOS * 128, C)
        v_sel = value[b].reshape(NBLK, 128, C)[h::2].reshape(NPOS * 128, C)
        in_maps.append(
            {
                "qt_in": np.ascontiguousarray(query[b].T.astype(BF16)),
                "kt_in": np.ascontiguousarray(k_sel.T.astype(BF16)),
                "v_in": np.ascontiguousarray(
                    np.concatenate([v_sel, pad], axis=1).astype(BF16)
                ),
                "m_in": masks[h],
            }
        )

    nc = _get_nc()
    res = run_bass_kernel_spmd(
        nc,
        in_maps,
        list(range(8)),
        trace=trace,
        trace_cores=trace_cores,
    )

    out = np.empty((B, S, C), np.float32)
    for b in range(B):
        o0 = np.asarray(res.results[2 * b]["o_out"]).astype(np.float64)
        o1 = np.asarray(res.results[2 * b + 1]["o_out"]).astype(np.float64)
        num = o0[:, :C] + o1[:, :C]
        den = o0[:, C] + o1[:, C]
        den = np.where(den == 0.0, 1.0, den)
        out[b] = (num / den[:, None]).astype(np.float32)
    return out.reshape(B, 64, 64, C), res


def kernel(query, key, value):
    out, _ = _run(query, key, value, trace=False)
    return out


# revision 36
# speedup vs baseline: 1.1549x; 1.0141x over previous
"""Trainium2 Bass kernel for PixelSNAIL-style strict-causal attention.

Problem: query/key/value [B=4, H=64, W=64, C=256] fp32.
  S = 4096 tokens per batch; scores = (Q K^T)/16 with strict causal mask
  (position i attends to j < i); out = softmax(scores) @ V (row 0 -> 0).

Strategy (8 NeuronCores):
  - 2 cores per batch: context-parallel split of the key/value blocks by
    parity (core h owns k-blocks h, h+2, ..., h+30). Every core runs the
    IDENTICAL program (SPMD) over all 32 query blocks of its batch.
  - No max-subtraction in softmax (scores ~ N(0,1), exp is safe), so
    per-core partial numerators/denominators combine exactly on host.
  - Q/K/V are pre-converted to bf16 on host (halves DMA, full-rate PE,
    hidden weight loads); exp output (probs) is bf16 too. PSUM stays f32.
  - Host pre-transposes Q and K (c-major) so no on-chip transposes are
    needed; V gets a ones-column appended so the softmax denominator
    accumulates in PSUM alongside the numerator.
  - Software pipelining: the PV matmuls of pair k are issued AFTER the
    scores matmuls of pair k+1, so the exp (ACT) of pair k overlaps PE
    work and the PE never stalls waiting on the activation.
  - The strict-causal/diagonal masking is data-driven: an additive mask
    applied on each q-slot's last position-pair, with the mask+exp chain
    split in halves so it fits inside the pipeline window.

Layout per core (b = core//2, h = core%2):
  qt_in [256, 4096] bf16 = Q[b]^T
  kt_in [256, 2048] bf16 = K[b][blocks h::2]^T
  v_in  [2048, 258] bf16 = V[b][blocks h::2] ++ ones column
  m_in  [128, 768]  f32  = additive mask for the last position-pair
  o_out [4096, 258] f32  = partial (numerator ++ denominator)
"""

import numpy as np
import ml_dtypes

BF16 = np.dtype(ml_dtypes.bfloat16)

B = 4
S = 4096          # 64*64 tokens per batch
C = 256
NBLK = 32         # 128-row k blocks per batch
NPOS = 16         # k blocks per core (parity split)
NSLOT = 8         # q slots of 512 rows
SCALE = 1.0 / 16.0
NEG = -1.0e30

_CACHE = {}


def _build_nc():
    import concourse.bacc as bacc
    import concourse.mybir as mybir
    import concourse.tile as tile

    F32 = mybir.dt.float32
    BF = mybir.dt.bfloat16

    nc = bacc.Bacc("TRN2", target_bir_lowering=False, debug=False, num_devices=8)
    qt_in = nc.dram_tensor("qt_in", [C, S], BF, kind="ExternalInput").ap()
    kt_in = nc.dram_tensor("kt_in", [C, NPOS * 128], BF, kind="ExternalInput").ap()
    v_in = nc.dram_tensor("v_in", [NPOS * 128, 258], BF, kind="ExternalInput").ap()
    m_in = nc.dram_tensor("m_in", [128, 512], F32, kind="ExternalInput").ap()
    o_out = nc.dram_tensor("o_out", [S, 258], BF, kind="ExternalOutput").ap()

    with tile.TileContext(nc) as tc:
        with (
            tc.tile_pool(name="const", bufs=1) as const,
            tc.tile_pool(name="pt", bufs=3) as ptp,
            tc.tile_pool(name="osb", bufs=2) as osbp,
            tc.tile_pool(name="st", bufs=2, space="PSUM") as stp,
            tc.tile_pool(name="op", bufs=4, space="PSUM") as opp,
        ):
            # PE warmup: tiny matmuls on memset data during the DMA
            # preamble so the HAM clock gate / p-state ramps before work.
            wu = const.tile([128, 64], BF, tag="wu")
            nc.gpsimd.memset(wu[:], 0.0)
            wu_ps = stp.tile([128, 512], F32, tag="st", name="wu_ps")
            for _ in range(60):
                nc.tensor.matmul(wu_ps[0:64, 0:64], lhsT=wu[:], rhs=wu[:],
                                 start=True, stop=True)

            qt = [const.tile([128, S], BF, tag=f"qt{c}", name=f"qt{c}") for c in range(2)]
            kt = [
                const.tile([128, NPOS * 128], BF, tag=f"kt{c}", name=f"kt{c}")
                for c in range(2)
            ]
            vsb = const.tile([128, NPOS * 258], BF, tag="v")
            mask = const.tile([128, 512], F32, tag="m")

            # DMA placement: q/k/mask inputs ride sync in consumption order;
            # k pair 0 + v ride gpsimd; the scalar(ACT) queue stays exp-only.
            def qt_dma(c, c0, c1):
                nc.sync.dma_start(qt[c][:, c0:c1], qt_in[c * 128:(c + 1) * 128, c0:c1])

            def kt_dma(c, c0, c1):
                nc.sync.dma_start(kt[c][:, c0:c1], kt_in[c * 128:(c + 1) * 128, c0:c1])

            def v_dma(pos, npos):
                nc.gpsimd.dma_start(
                    vsb[:, pos * 258:(pos + npos) * 258].rearrange(
                        "p (t v) -> p t v", t=npos
                    ),
                    v_in[pos * 128:(pos + npos) * 128, :].rearrange(
                        "(t p) v -> p t v", p=128
                    ),
                )

            # gpsimd queue: k pair 0 first (gates the first real matmul),
            # then v in consumption order.
            for c in range(2):
                nc.gpsimd.dma_start(kt[c][:, 0:256], kt_in[c * 128:(c + 1) * 128, 0:256])
            v_dma(0, 2)
            v_dma(2, 2)
            v_dma(4, 4)
            v_dma(8, 8)
            # sync queue: slot-7 queries first, then k pairs, mask, rest of q.
            for c in range(2):
                qt_dma(c, 7 * 512, 8 * 512)
            for c in range(2):
                kt_dma(c, 256, 1024)                  # k pairs 1-3
            for c in range(2):
                kt_dma(c, 1024, 2048)                 # k pairs 4-7
            nc.sync.dma_start(mask[:], m_in[:])
            for c in range(2):
                qt_dma(c, 6 * 512, 7 * 512)
            for c in range(2):
                qt_dma(c, 4 * 512, 6 * 512)
            for c in range(2):
                qt_dma(c, 0, 4 * 512)

            o_ps = {}
            # Pairs awaiting PV issue: [p, t, pt tile, age]. Non-diag pairs
            # flush one pair after their scores (software pipelining); diag
            # pairs flush after two, giving the mask+exp chain extra slack.
            pending = []

            def issue_pv(p, t, pt):
                last = t == p
                first_it = t == 0
                for jp in range(2):
                    for qs in range(4):
                        if last and jp == 1 and qs < 2:
                            continue
                        loff = jp * 512 + qs * 128
                        if last and jp == 1:
                            loff = 512 + (qs - 2) * 128
                        nc.tensor.matmul(
                            o_ps[p][qs][:],
                            lhsT=pt[:, loff:loff + 128],
                            rhs=vsb[:, (2 * t + jp) * 258:(2 * t + jp + 1) * 258],
                            start=(first_it and jp == 0),
                            stop=(last and jp == (0 if qs < 2 else 1)),
                        )
                if last:
                    # Slot p complete: drain its PSUM O tiles into one bf16
                    # staging tile (GPSIMD cannot read PSUM on TRN2), then a
                    # single DMA. On the final slots the scalar engine (idle
                    # by then) takes half the copies to shorten the tail.
                    ob = osbp.tile([128, 4 * 258], BF, tag="ob", name=f"ob{p}")
                    for qs in range(4):
                        dst = ob[:, qs * 258:(qs + 1) * 258]
                        if p <= 1 and qs >= 2:
                            nc.scalar.copy(dst, o_ps[p][qs][:])
                        else:
                            nc.vector.tensor_copy(dst, o_ps[p][qs][:])
                    nc.sync.dma_start(
                        o_out[p * 512:(p + 1) * 512, :].rearrange(
                            "(q part) v -> part q v", part=128
                        ),
                        ob[:].rearrange("part (q v) -> part q v", q=4),
                    )

            for p in range(NSLOT - 1, -1, -1):
                o_ps[p] = [
                    opp.tile([128, 258], F32, tag="o", name=f"o_ps{p}_{qs}")
                    for qs in range(4)
                ]
                for t in range(p + 1):
                    last = t == p
                    width = 768 if last else 1024
                    st = stp.tile([128, 1024], F32, tag="st", name=f"st{p}_{t}")
                    if p == NSLOT - 1 and t < 3:
                        # keep the PE clock gate open while DMA-paced
                        for _ in range(12 if t == 0 else 6):
                            nc.tensor.matmul(
                                st[0:64, 0:64], lhsT=wu[:], rhs=wu[:],
                                start=True, stop=True,
                            )
                    # On the diag pair, compute the masked jp1 half first so
                    # its mask+exp chain starts while jp0's scores still run.
                    for jp in ((1, 0) if last else (0, 1)):
                        pos = 2 * t + jp
                        qoff = p * 512 + (256 if (last and jp == 1) else 0)
                        n = 256 if (last and jp == 1) else 512
                        for c in range(2):
                            nc.tensor.matmul(
                                st[:, jp * 512:jp * 512 + n],
                                lhsT=kt[c][:, pos * 128:(pos + 1) * 128],
                                rhs=qt[c][:, qoff:qoff + n],
                                start=(c == 0),
                                stop=(c == 1),
                            )
                    pt = ptp.tile([128, 1024], BF, tag="pt", name=f"pt{p}_{t}")
                    if last:
                        # Only 256 columns per jp-half can actually be masked
                        # (qs-blocks at/past the diagonal); mask just those so
                        # the mask+exp chain stays inside the pipeline window.
                        nc.vector.tensor_tensor(
                            st[:, 512:768], st[:, 512:768], mask[:, 256:512],
                            mybir.AluOpType.add,
                        )
                        nc.scalar.activation(
                            pt[:, 512:768], st[:, 512:768],
                            mybir.ActivationFunctionType.Exp, scale=SCALE,
                        )
                        nc.vector.tensor_tensor(
                            st[:, 0:256], st[:, 0:256], mask[:, 0:256],
                            mybir.AluOpType.add,
                        )
                        nc.scalar.activation(
                            pt[:, 0:512], st[:, 0:512],
                            mybir.ActivationFunctionType.Exp, scale=SCALE,
                        )
                    else:
                        nc.scalar.activation(
                            pt[:, :width], st[:, :width],
                            mybir.ActivationFunctionType.Exp, scale=SCALE,
                        )
                    for q in pending:
                        q[3] += 1
                    if pending and pending[0][3] >= (2 if pending[0][1] == pending[0][0] else 1):
                        issue_pv(*pending.pop(0)[:3])
                    pending.append([p, t, pt, 0])
            for q in pending:
                issue_pv(*q[:3])
    nc.compile()
    return nc


def _get_nc():
    if "nc" not in _CACHE:
        _CACHE["nc"] = _build_nc()
    return _CACHE["nc"]


def _make_masks():
    """Additive masks [128, 512] for the last position-pair of each slot.

    Only the q-sub-blocks at/past the diagonal can be masked: st columns
    0:256 (jp=0, qs 0-1) and 512:768 (jp=1, qs 2-3). On the last pair t=p,
    position jp holds k-block 4p + 2*jp + h vs q-sub-block 4p + qs:
      block <  qblock -> fully allowed (0)
      block == qblock -> strict lower-triangular (allowed iff q_local > k_local)
      block >  qblock -> fully blocked (NEG)
    """
    k_loc = np.arange(128)[:, None]
    q_loc = np.arange(128)[None, :]
    strict = np.where(q_loc > k_loc, 0.0, NEG).astype(np.float32)
    zeros = np.zeros((128, 128), np.float32)
    blocked = np.full((128, 128), NEG, np.float32)
    masks = []
    for h in range(2):
        chunks = []
        for jp, qs_list in ((0, (0, 1)), (1, (2, 3))):
            rel = 2 * jp + h  # k-block offset relative to 4p
            for qs in qs_list:
                if rel < qs:
                    chunks.append(zeros)
                elif rel == qs:
                    chunks.append(strict)
                else:
                    chunks.append(blocked)
        masks.append(np.concatenate(chunks, axis=1))
    return masks


def _run(query, key, value, trace=False, trace_cores=None):
    from concourse.bass_utils import run_bass_kernel_spmd

    query = np.ascontiguousarray(np.asarray(query, dtype=np.float32)).reshape(B, S, C)
    key = np.ascontiguousarray(np.asarray(key, dtype=np.float32)).reshape(B, S, C)
    value = np.ascontiguousarray(np.asarray(value, dtype=np.float32)).reshape(B, S, C)

    masks = _make_masks()
    pad = np.zeros((NPOS * 128, 2), np.float32)
    pad[:, 0] = 1.0
    in_maps = []
    for core in range(8):
        b, h = core // 2, core % 2
        k_sel = key[b].reshape(NBLK, 128, C)[h::2].reshape(NPOS * 128, C)
        v_sel = value[b].reshape(NBLK, 128, C)[h::2].reshape(NPOS * 128, C)
        in_maps.append(
            {
                "qt_in": np.ascontiguousarray(query[b].T.astype(BF16)),
                "kt_in": np.ascontiguousarray(k_sel.T.astype(BF16)),
                "v_in": np.ascontiguousarray(
                    np.concatenate([v_sel, pad], axis=1).astype(BF16)
                ),
                "m_in": masks[h],
            }
        )

    nc = _get_nc()
    res = run_bass_kernel_spmd(
        nc,
        in_maps,
        list(range(8)),
        trace=trace,
        trace_cores=trace_cores,
    )

    out = np.empty((B, S, C), np.float32)
    for b in range(B):
        o0 = np.asarray(res.results[2 * b]["o_out"]).astype(np.float64)
        o1 = np.asarray(res.results[2 * b + 1]["o_out"]).astype(np.float64)
        num = o0[:, :C] + o1[:, :C]
        den = o0[:, C] + o1[:, C]
        den = np.where(den == 0.0, 1.0, den)
        out[b] = (num / den[:, None]).astype(np.float32)
    return out.reshape(B, 64, 64, C), res


def kernel(query, key, value):
    out, _ = _run(query, key, value, trace=False)
    return out


# revision 42
# speedup vs baseline: 1.1945x; 1.0343x over previous
"""Trainium2 Bass kernel for PixelSNAIL-style strict-causal attention.

Problem: query/key/value [B=4, H=64, W=64, C=256] fp32.
  S = 4096 tokens per batch; scores = (Q K^T)/16 with strict causal mask
  (position i attends to j < i); out = softmax(scores) @ V (row 0 -> 0).

Strategy (8 NeuronCores):
  - 2 cores per batch: context-parallel split of the key/value blocks by
    parity (core h owns k-blocks h, h+2, ..., h+30). Every core runs the
    IDENTICAL program (SPMD) over all 32 query blocks of its batch.
  - No max-subtraction in softmax (scores ~ N(0,1), exp is safe), so
    per-core partial numerators/denominators combine exactly on host.
  - Q/K/V are pre-converted to bf16 on host (halves DMA, full-rate PE,
    hidden weight loads); exp output (probs) is bf16 too. PSUM stays f32.
  - Host pre-transposes Q and K (c-major) so no on-chip transposes are
    needed; V gets a ones-column appended so the softmax denominator
    accumulates in PSUM alongside the numerator.
  - Software pipelining: the PV matmuls of pair k are issued AFTER the
    scores matmuls of pair k+1, so the exp (ACT) of pair k overlaps PE
    work and the PE never stalls waiting on the activation.
  - The strict-causal/diagonal masking is data-driven: an additive mask
    applied on each q-slot's last position-pair, with the mask+exp chain
    split in halves so it fits inside the pipeline window.

Layout per core (b = core//2, h = core%2):
  qt_in [256, 4096] bf16 = Q[b]^T
  kt_in [256, 2048] bf16 = K[b][blocks h::2]^T
  v_in  [2048, 258] bf16 = V[b][blocks h::2] ++ ones column
  m_in  [128, 768]  f32  = additive mask for the last position-pair
  o_out [4096, 258] f32  = partial (numerator ++ denominator)
"""

import numpy as np
import ml_dtypes

BF16 = np.dtype(ml_dtypes.bfloat16)

B = 4
S = 4096          # 64*64 tokens per batch
C = 256
NBLK = 32         # 128-row k blocks per batch
NPOS = 16         # k blocks per core (parity split)
NSLOT = 8         # q slots of 512 rows
SCALE = 1.0 / 16.0
NEG = -1.0e30

_CACHE = {}


def _build_nc():
    import concourse.bacc as bacc
    import concourse.mybir as mybir
    import concourse.tile as tile

    F32 = mybir.dt.float32
    BF = mybir.dt.bfloat16

    nc = bacc.Bacc("TRN2", target_bir_lowering=False, debug=False, num_devices=8)
    qt_in = nc.dram_tensor("qt_in", [C, S], BF, kind="ExternalInput").ap()
    kt_in = nc.dram_tensor("kt_in", [C, NPOS * 128], BF, kind="ExternalInput").ap()
    v_in = nc.dram_tensor("v_in", [NPOS * 128, 258], BF, kind="ExternalInput").ap()
    m_in = nc.dram_tensor("m_in", [128, 512], BF, kind="ExternalInput").ap()
    o_out = nc.dram_tensor("o_out", [S, 258], BF, kind="ExternalOutput").ap()

    with tile.TileContext(nc) as tc:
        with (
            tc.tile_pool(name="const", bufs=1) as const,
            tc.tile_pool(name="pt", bufs=3) as ptp,
            tc.tile_pool(name="osb", bufs=2) as osbp,
            tc.tile_pool(name="st", bufs=2, space="PSUM") as stp,
            tc.tile_pool(name="op", bufs=4, space="PSUM") as opp,
        ):
            # PE warmup: tiny matmuls on memset data during the DMA
            # preamble so the HAM clock gate / p-state ramps before work.
            wu = const.tile([128, 64], BF, tag="wu")
            nc.gpsimd.memset(wu[:], 0.0)
            wu_ps = stp.tile([128, 512], F32, tag="st", name="wu_ps")
            for _ in range(60):
                nc.tensor.matmul(wu_ps[0:64, 0:64], lhsT=wu[:], rhs=wu[:],
                                 start=True, stop=True)

            qt = [const.tile([128, S], BF, tag=f"qt{c}", name=f"qt{c}") for c in range(2)]
            kt = [
                const.tile([128, NPOS * 128], BF, tag=f"kt{c}", name=f"kt{c}")
                for c in range(2)
            ]
            vsb = const.tile([128, NPOS * 258], BF, tag="v")
            mask = const.tile([128, 512], BF, tag="m")

            # DMA placement: q/k/mask inputs ride sync in consumption order;
            # k pair 0 + v ride gpsimd; the scalar(ACT) queue stays exp-only.
            def qt_dma(c, c0, c1):
                nc.sync.dma_start(qt[c][:, c0:c1], qt_in[c * 128:(c + 1) * 128, c0:c1])

            def kt_dma(c, c0, c1):
                nc.sync.dma_start(kt[c][:, c0:c1], kt_in[c * 128:(c + 1) * 128, c0:c1])

            def v_dma(pos, npos):
                nc.gpsimd.dma_start(
                    vsb[:, pos * 258:(pos + npos) * 258].rearrange(
                        "p (t v) -> p t v", t=npos
                    ),
                    v_in[pos * 128:(pos + npos) * 128, :].rearrange(
                        "(t p) v -> p t v", p=128
                    ),
                )

            # gpsimd queue: k pair 0 first (gates the first real matmul),
            # then v in consumption order.
            for c in range(2):
                nc.gpsimd.dma_start(kt[c][:, 0:256], kt_in[c * 128:(c + 1) * 128, 0:256])
            v_dma(0, 2)
            v_dma(2, 2)
            v_dma(4, 4)
            v_dma(8, 8)
            # sync queue: slot-7 queries first, then k pairs, mask, rest of q.
            for c in range(2):
                qt_dma(c, 7 * 512, 8 * 512)
            for c in range(2):
                kt_dma(c, 256, 1024)                  # k pairs 1-3
            for c in range(2):
                kt_dma(c, 1024, 2048)                 # k pairs 4-7
            nc.sync.dma_start(mask[:], m_in[:])
            for c in range(2):
                qt_dma(c, 6 * 512, 7 * 512)
            for c in range(2):
                qt_dma(c, 4 * 512, 6 * 512)
            for c in range(2):
                qt_dma(c, 0, 4 * 512)

            o_ps = {}
            # Pairs awaiting PV issue: [p, t, pt tile, age]. Non-diag pairs
            # flush one pair after their scores (software pipelining); diag
            # pairs flush after two, giving the mask+exp chain extra slack.
            pending = []

            def issue_pv(p, t, pt):
                last = t == p
                first_it = t == 0
                for jp in range(2):
                    for qs in range(4):
                        if last and jp == 1 and qs < 2:
                            continue
                        loff = jp * 512 + qs * 128
                        if last and jp == 1:
                            loff = 512 + (qs - 2) * 128
                        nc.tensor.matmul(
                            o_ps[p][qs][:],
                            lhsT=pt[:, loff:loff + 128],
                            rhs=vsb[:, (2 * t + jp) * 258:(2 * t + jp + 1) * 258],
                            start=(first_it and jp == 0),
                            stop=(last and jp == (0 if qs < 2 else 1)),
                        )
                if last:
                    # Slot p complete: drain its PSUM O tiles into one bf16
                    # staging tile (GPSIMD cannot read PSUM on TRN2), then a
                    # single DMA. On the final slots the scalar engine (idle
                    # by then) takes half the copies to shorten the tail.
                    ob = osbp.tile([128, 4 * 258], BF, tag="ob", name=f"ob{p}")
                    for qs in range(4):
                        dst = ob[:, qs * 258:(qs + 1) * 258]
                        if p <= 2 and qs >= 2:
                            nc.scalar.copy(dst, o_ps[p][qs][:])
                        else:
                            nc.vector.tensor_copy(dst, o_ps[p][qs][:])
                    nc.sync.dma_start(
                        o_out[p * 512:(p + 1) * 512, :].rearrange(
                            "(q part) v -> part q v", part=128
                        ),
                        ob[:].rearrange("part (q v) -> part q v", q=4),
                    )

            for p in range(NSLOT - 1, -1, -1):
                o_ps[p] = [
                    opp.tile([128, 258], F32, tag="o", name=f"o_ps{p}_{qs}")
                    for qs in range(4)
                ]
                for t in range(p + 1):
                    last = t == p
                    width = 768 if last else 1024
                    st = stp.tile([128, 1024], F32, tag="st", name=f"st{p}_{t}")
                    if p == NSLOT - 1 and t < 3:
                        # keep the PE clock gate open while DMA-paced
                        for _ in range(12 if t == 0 else 6):
                            nc.tensor.matmul(
                                st[0:64, 0:64], lhsT=wu[:], rhs=wu[:],
                                start=True, stop=True,
                            )
                    for jp in range(2):
                        pos = 2 * t + jp
                        qoff = p * 512 + (256 if (last and jp == 1) else 0)
                        n = 256 if (last and jp == 1) else 512
                        for c in range(2):
                            nc.tensor.matmul(
                                st[:, jp * 512:jp * 512 + n],
                                lhsT=kt[c][:, pos * 128:(pos + 1) * 128],
                                rhs=qt[c][:, qoff:qoff + n],
                                start=(c == 0),
                                stop=(c == 1),
                            )
                    pt = ptp.tile([128, 1024], BF, tag="pt", name=f"pt{p}_{t}")
                    if last:
                        # Multiplicative masking AFTER a single full-width
                        # exp: keeps the mask off the St (PSUM) critical path
                        # so the next scores reuse St as early as possible.
                        # Only 256 columns per jp-half are maskable; the two
                        # multiplies run on gpsimd and vector in parallel.
                        nc.scalar.activation(
                            pt[:, 0:width], st[:, 0:width],
                            mybir.ActivationFunctionType.Exp, scale=SCALE,
                        )
                        nc.gpsimd.tensor_tensor(
                            pt[:, 0:256], pt[:, 0:256], mask[:, 0:256],
                            mybir.AluOpType.mult,
                        )
                        nc.vector.tensor_tensor(
                            pt[:, 512:768], pt[:, 512:768], mask[:, 256:512],
                            mybir.AluOpType.mult,
                        )
                    else:
                        nc.scalar.activation(
                            pt[:, :width], st[:, :width],
                            mybir.ActivationFunctionType.Exp, scale=SCALE,
                        )
                    for q in pending:
                        q[3] += 1
                    if pending and pending[0][3] >= (2 if pending[0][1] == pending[0][0] else 1):
                        issue_pv(*pending.pop(0)[:3])
                    pending.append([p, t, pt, 0])
            for q in pending:
                issue_pv(*q[:3])
    nc.compile()
    return nc


def _get_nc():
    if "nc" not in _CACHE:
        _CACHE["nc"] = _build_nc()
    return _CACHE["nc"]


def _make_masks():
    """Multiplicative 0/1 masks [128, 512] for each slot's last position-pair.

    Only the q-sub-blocks at/past the diagonal can be masked: pt columns
    0:256 (jp=0, qs 0-1) and 512:768 (jp=1, qs 2-3). On the last pair t=p,
    position jp holds k-block 4p + 2*jp + h vs q-sub-block 4p + qs:
      block <  qblock -> fully allowed (1)
      block == qblock -> strict lower-triangular (1 iff q_local > k_local)
      block >  qblock -> fully blocked (0)
    """
    k_loc = np.arange(128)[:, None]
    q_loc = np.arange(128)[None, :]
    strict = (q_loc > k_loc).astype(np.float32)
    ones = np.ones((128, 128), np.float32)
    blocked = np.zeros((128, 128), np.float32)
    masks = []
    for h in range(2):
        chunks = []
        for jp, qs_list in ((0, (0, 1)), (1, (2, 3))):
            rel = 2 * jp + h  # k-block offset relative to 4p
            for qs in qs_list:
                if rel < qs:
                    chunks.append(ones)
                elif rel == qs:
                    chunks.append(strict)
                else:
                    chunks.append(blocked)
        masks.append(np.concatenate(chunks, axis=1).astype(BF16))
    return masks


def _run(query, key, value, trace=False, trace_cores=None):
    from concourse.bass_utils import run_bass_kernel_spmd

    query = np.ascontiguousarray(np.asarray(query, dtype=np.float32)).reshape(B, S, C)
    key = np.ascontiguousarray(np.asarray(key, dtype=np.float32)).reshape(B, S, C)
    value = np.ascontiguousarray(np.asarray(value, dtype=np.float32)).reshape(B, S, C)

    masks = _make_masks()
    pad = np.zeros((NPOS * 128, 2), np.float32)
    pad[:, 0] = 1.0
    in_maps = []
    for core in range(8):
        b, h = core // 2, core % 2
        k_sel = key[b].reshape(NBLK, 128, C)[h::2].reshape(NPOS * 128, C)
        v_sel = value[b].reshape(NBLK, 128, C)[h::2].reshape(NPOS * 128, C)
        in_maps.append(
            {
                "qt_in": np.ascontiguousarray(query[b].T.astype(BF16)),
                "kt_in": np.ascontiguousarray(k_sel.T.astype(BF16)),
                "v_in": np.ascontiguousarray(
                    np.concatenate([v_sel, pad], axis=1).astype(BF16)
                ),
                "m_in": masks[h],
            }
        )

    nc = _get_nc()
    res = run_bass_kernel_spmd(
        nc,
        in_maps,
        list(range(8)),
        trace=trace,
        trace_cores=trace_cores,
    )

    out = np.empty((B, S, C), np.float32)
    for b in range(B):
        o0 = np.asarray(res.results[2 * b]["o_out"]).astype(np.float64)
        o1 = np.asarray(res.results[2 * b + 1]["o_out"]).astype(np.float64)
        num = o0[:, :C] + o1[:, :C]
        den = o0[:, C] + o1[:, C]
        den = np.where(den == 0.0, 1.0, den)
        out[b] = (num / den[:, None]).astype(np.float32)
    return out.reshape(B, 64, 64, C), res


def kernel(query, key, value):
    out, _ = _run(query, key, value, trace=False)
    return out
